# revision 9
# baseline (speedup 1.0000x reference)
"""Causal MHA (B=4, S=4096, D=64, scale=1/sqrt(S)) on 8 trn2 NeuronCores.

v4: dual-engine exp + fp8 DoubleRow AV + depth-3 chunk pipeline.

v4 over v3: measured engine-cost constants for the greedy balance (ACT
1110ns, DVE 1224ns per 1024-col chunk), slot-1 mask multiply moved to the
idle GpSimd engine, drains merged per slot-pair ([65,512], 4 copies + 4
DMAs instead of 8), finer first-KTP DMA split for faster start.

Strategy (identical SPMD program on all 8 cores; per-core data differs):
  - Sharding: core c -> batch b=c%4, half=c//4; 8 query slots of 256 rows,
    slot m covers q-block j=2m-1-half, iterates T=4m k-tiles of 128 keys;
    scoresT layout [k, q]; ones-column denominator; host divides.
  - Chunks of FOUR k-tiles, PSUM scores [128,1024] fp32 = 2 banks, THREE
    buffers (6 banks) + 2 AV accumulator banks = 8.  Depth-3 keeps the QK
    refill off the exp engines' critical path (v2's depth-2 stalled both
    engines every other chunk, PE went HAM-cold 15.6-29us).
  - QK: bf16 paired matmuls (adjacent k-tiles in partition halves 0/1,
    concurrent in disjoint PE row groups; measured 120ns/pair warm),
    perm [0,2,1,3] puts each pair in different PSUM banks.
  - exp (elementwise wall, 36864 cols/lane) split across TWO engines:
      ACT: real exp (scale folded), direct fp8e4 output (bit-exact RNE,
           probe-verified), ~925ns/chunk.
      DVE: Schraudolph bit-trick i8 = RNE((s + B8)*M8): the int8 bits ARE
           e4m3 of exp(s/64) (~2.5%/elem sawtooth, cancels in the softmax
           ratio).  Masked tail chunks fold the causal mask in FREE via
           scalar_tensor_tensor (s + B8) * Mtile, Mtile in {M8, 0}.
           ~1192ns/chunk.  fp32->int8 convert is exact RNE (probe).
    Greedy balance by modeled ns; tails forced to DVE; drains balanced.
  - AV: fp8e4 DoubleRow, TWO k-tiles per matmul ([128,2,65] x [128,2,256],
    V tiles padded to stride 80 for the %16 pair-step ISA rule).  Measured
    109ns/pair with LDWEIGHTS fully hidden.  exp tiles are [128,2,2,256]
    so pair i is the clean slice [:, :, i, :].
  - Precision: rows q<512 (slot 1) have concentrated attention (softmax
    over <=2 tiles): sawtooth/fp8 errors don't cancel there (v2 measured
    4e-2 at row 1).  Slot 1 uses ACT real exp -> bf16, a DVE 0/1-mask
    multiply, and bf16 AV.  Everything else fp8 (worst other block 7e-3
    abs vs gate 7e-2).
  - ACT Exp table preloaded during input DMAs (1.3us off critical path).
  - PE warm-up burst (N=512 matmuls) trips the HAM clock gate to 2.4GHz
    while DMAs land; steady-state PE duty ~45% keeps it warm.
Modeled walls: ACT ~20.5us, DVE ~20.7us, PE ~16.5us busy.
"""

import sys

sys.path.insert(0, "/opt/trn_rl_repo")

import numpy as np
import ml_dtypes

B, S, D = 4, 4096, 64
NCORES = 8
NSLOTS = 8          # query slots per core, 256 queries each
QS = 256            # queries per slot
KT_TILE = 128       # keys per k-tile
NKT = S // KT_TILE  # 32 k-tiles per batch
VW = 80             # padded V-tile width (65 data cols; 80 for %16 pair step)
CS = 4              # k-tiles per chunk
BF16 = ml_dtypes.bfloat16
E4M3 = ml_dtypes.float8_e4m3

_COMPILED = None

WARM_MMS = 10       # PE warm-up matmuls (N=512) to trip the HAM clock gate
_CACHE_BUST = 8     # bump to force a NEFF recompile

# Schraudolph constants: exp(s/64) ~= bitcast(round((s + Bc)*Mc)).
# Mc is the bf16-representable slope; -0.3 centers the piecewise-linear
# error (fitted offline, numerics_sim.py; RNE convert probe-verified).
_LN2 = float(np.log(2.0))
M8 = float(BF16(8.0 / (64.0 * _LN2)))       # 0.180664
B8 = (56.0 - 0.3) / M8

PERM4 = [0, 2, 1, 3]   # tile r -> PSUM block; QK pairs hit different banks


def _act_ns(fd):
    # measured on HW at FD=1024 (v3 trace): 1110ns
    return (fd + 310.0) / 1.2


def _dve_ns(fd):
    # measured on HW at FD=1024 (v3 trace): 1224ns
    return (fd + 155.0) / 0.96


def _build_program():
    import concourse.bacc as bacc
    import concourse.tile as tile
    import concourse.mybir as mybir

    F32 = mybir.dt.float32
    MBF16 = mybir.dt.bfloat16
    I8 = mybir.dt.int8
    F8 = mybir.dt.float8e4
    EXPF = mybir.ActivationFunctionType.Exp
    ALU = mybir.AluOpType
    DR = mybir.MatmulPerfMode.DoubleRow

    nc = bacc.Bacc("TRN2", target_bir_lowering=False, debug=False, num_devices=NCORES)

    ktp = nc.dram_tensor("ktp", [128, 16 * 128], MBF16, kind="ExternalInput").ap()
    qtd = nc.dram_tensor("qtd", [128, NSLOTS * QS], MBF16, kind="ExternalInput").ap()
    va8 = nc.dram_tensor("va8", [128, NKT * VW], F8, kind="ExternalInput").ap()
    va16 = nc.dram_tensor("va16", [128, 4 * 65], MBF16, kind="ExternalInput").ap()
    mt4_8 = nc.dram_tensor("mt4_8", [128, 4 * QS], MBF16, kind="ExternalInput").ap()
    m01 = nc.dram_tensor("m01", [128, 4 * QS], MBF16, kind="ExternalInput").ap()
    ot = nc.dram_tensor("ot", [65, NSLOTS * QS], F32, kind="ExternalOutput").ap()

    with tile.TileContext(nc) as tc:
        with (
            tc.tile_pool(name="ins", bufs=1) as ins,
            tc.tile_pool(name="work", bufs=5) as work,
            tc.tile_pool(name="outs", bufs=2) as outs,
            tc.tile_pool(name="ps", bufs=1, space="PSUM") as ps,
            tc.tile_pool(name="pso", bufs=1, space="PSUM") as pso,
        ):
            ktp_sb = ins.tile([128, 16 * 128], MBF16)
            qtd_sb = ins.tile([128, NSLOTS * QS], MBF16)
            va8_sb = ins.tile([128, NKT, VW], F8)
            va16_sb = ins.tile([128, 4, 65], MBF16)
            mt4_8_sb = ins.tile([128, 4 * QS], MBF16)
            m01_sb = ins.tile([128, 4 * QS], MBF16)
            warm_sb = ins.tile([128, 512], MBF16)
            tbl_sb = ins.tile([128, 8], F8)

            # PE warm-up burst: trips the HAM clock gate to 2.4 GHz while
            # the input DMAs stream.  Also preload the ACT Exp spline table
            # (1.3us) so the first real exp doesn't pay it.
            nc.gpsimd.memset(warm_sb, 0.25)
            nc.scalar.activation(tbl_sb, warm_sb[:, :8], EXPF, scale=1.0 / 64.0)
            p_warm = ps.tile([128, CS * QS], F32, tag="scores", bufs=3)
            for _ in range(WARM_MMS):
                nc.tensor.matmul(
                    p_warm[:, :512],
                    warm_sb[:, :128],
                    warm_sb[:, :512],
                    start=True,
                    stop=True,
                )

            # Input DMAs: split + ordered by first consumption.
            nc.sync.dma_start(out=qtd_sb[:, 7 * QS :], in_=qtd[:, 7 * QS :])  # slot 8
            nc.sync.dma_start(out=ktp_sb[:, :256], in_=ktp[:, :256])          # k-tiles 0-3
            nc.sync.dma_start(out=ktp_sb[:, 256:512], in_=ktp[:, 256:512])    # k-tiles 4-7
            nc.sync.dma_start(out=va8_sb[:, :8, :], in_=va8[:, : 8 * VW])     # v-tiles 0-7
            nc.sync.dma_start(out=ktp_sb[:, 512:896], in_=ktp[:, 512:896])    # k-tiles 8-13
            nc.sync.dma_start(out=ktp_sb[:, 896:], in_=ktp[:, 896:])
            nc.sync.dma_start(out=va8_sb[:, 8:, :], in_=va8[:, 8 * VW :])
            nc.sync.dma_start(out=mt4_8_sb, in_=mt4_8)
            nc.sync.dma_start(out=qtd_sb[:, 4 * QS : 7 * QS], in_=qtd[:, 4 * QS : 7 * QS])
            nc.sync.dma_start(out=qtd_sb[:, : 4 * QS], in_=qtd[:, : 4 * QS])
            nc.sync.dma_start(out=va16_sb[:, :, :], in_=va16)
            nc.sync.dma_start(out=m01_sb, in_=m01)

            eng_ns = {"act": 0.0, "dve": 0.0}  # modeled busy time

            def drain(po_pair, qlo_pair):
                # one [65,512] copy + DMA per slot PAIR
                o_sb = outs.tile([65, 2 * QS], F32, tag="drain")
                if eng_ns["act"] + _act_ns(2 * QS) <= eng_ns["dve"] + _dve_ns(2 * QS):
                    nc.scalar.copy(o_sb, po_pair)
                    eng_ns["act"] += _act_ns(2 * QS)
                else:
                    nc.vector.tensor_copy(o_sb, po_pair)
                    eng_ns["dve"] += _dve_ns(2 * QS)
                nc.sync.dma_start(out=ot[:, qlo_pair : qlo_pair + 2 * QS], in_=o_sb)

            # AV jobs deferred by one chunk so exp->AV is never on the
            # ACT/DVE critical path.  job: (ready_g, emit_fn)
            av_jobs = []

            def emit_ready(now):
                rest = []
                for ready, fn in av_jobs:
                    if ready <= now:
                        fn()
                    else:
                        rest.append((ready, fn))
                av_jobs[:] = rest

            FD = CS * QS
            g = 0  # global chunk counter
            po_pair = None
            prev_eng = [None]  # engine of the previous chunk, for interleaving
            SLOT_ORDER = [8, 7, 6, 5, 4, 3, 1, 2]
            for oi, m in enumerate(SLOT_ORDER):
                qlo = (m - 1) * QS
                T = 4 * m
                slot1 = m == 1
                if oi % 2 == 0:  # first slot of a pair
                    po_pair = pso.tile([65, 2 * QS], F32, tag="avout", bufs=2)
                    pair_lo = min(m, SLOT_ORDER[oi + 1])
                # within the pair bank: smaller-m slot -> cols 0:256 (matches
                # the contiguous ot range of the pair)
                p_out = po_pair[:, :QS] if m == pair_lo else po_pair[:, QS : 2 * QS]
                last_of_pair = oi % 2 == 1
                pair_tile = po_pair
                qlo_pair = (pair_lo - 1) * QS if last_of_pair else None
                nchunks = T // CS
                for ci in range(nchunks):
                    base = ci * CS
                    tail = ci == nchunks - 1
                    p_sc = ps.tile([128, CS * QS], F32, tag="scores", bufs=3)
                    for j in range(CS):
                        t = base + j
                        h = t % 2
                        u = t // 2
                        nc.tensor.matmul(
                            p_sc[:, PERM4[j] * QS : PERM4[j] * QS + QS],
                            ktp_sb[64 * h : 64 * h + 64, 128 * u : 128 * u + 128],
                            qtd_sb[64 * h : 64 * h + 64, qlo : qlo + QS],
                            start=True,
                            stop=True,
                        )

                    # ---- exp + AV jobs ----
                    if slot1:
                        # concentrated-attention rows: real exp, bf16 AV.
                        e16 = work.tile([128, 2, 2, QS], MBF16, tag="e16")
                        nc.scalar.activation(
                            e16[:, :, :, :], p_sc[:, :FD], EXPF, scale=1.0 / 64.0
                        )
                        eng_ns["act"] += _act_ns(FD)
                        prev_eng[0] = "act"
                        e16m = work.tile([128, 2, 2, QS], MBF16, tag="e16m")
                        nc.gpsimd.tensor_mul(
                            e16m[:, :, :, :], e16[:, :, :, :], m01_sb
                        )
                        for j in range(CS):
                            t = base + j
                            r, i = j % 2, j // 2

                            def emit16(t=t, r=r, i=i, e=e16m, po=p_out, T=T,
                                       fin=last_of_pair, pt=pair_tile, qp=qlo_pair):
                                nc.tensor.matmul(
                                    po,
                                    va16_sb[:, t, :],
                                    e[:, r, i, :],
                                    start=(t == 0),
                                    stop=(t == T - 1),
                                )
                                if t == T - 1 and fin:
                                    drain(pt, qp)

                            av_jobs.append((g + 3, emit16))
                    else:
                        if tail:
                            # masked fp8 Schraudolph on DVE, mask fused
                            e8 = work.tile([128, 2, 2, QS], I8, tag="e8i")
                            nc.vector.scalar_tensor_tensor(
                                e8[:, :, :, :], p_sc[:, :FD], float(B8),
                                mt4_8_sb, ALU.add, ALU.mult,
                            )
                            eng_ns["dve"] += _dve_ns(FD)
                            prev_eng[0] = "dve"
                            nat = False
                        elif m == 2 or (eng_ns["act"] + _act_ns(FD)
                              + (600.0 if prev_eng[0] == "act" else 0.0)) <= (
                              eng_ns["dve"] + _dve_ns(FD)
                              + (600.0 if prev_eng[0] == "dve" else 0.0)):
                            e8 = work.tile([128, 2, 2, QS], F8, tag="e8a")
                            nc.scalar.activation(
                                e8[:, :, :, :], p_sc[:, :FD], EXPF, scale=1.0 / 64.0
                            )
                            eng_ns["act"] += _act_ns(FD)
                            prev_eng[0] = "act"
                            nat = True
                        else:
                            e8 = work.tile([128, 2, 2, QS], I8, tag="e8i")
                            nc.vector.tensor_scalar(
                                e8[:, :, :, :], p_sc[:, :FD], float(B8), float(M8),
                                ALU.add, ALU.mult,
                            )
                            eng_ns["dve"] += _dve_ns(FD)
                            prev_eng[0] = "dve"
                            nat = False

                        for i in range(2):
                            t = base + 2 * i

                            def emit8(t=t, i=i, e=e8, nat=nat, po=p_out, T=T,
                                      fin=last_of_pair, pt=pair_tile, qp=qlo_pair):
                                rhs = e[:, :, i, :]
                                if not nat:
                                    rhs = rhs.bitcast(F8)
                                nc.tensor.matmul(
                                    po,
                                    va8_sb[:, t : t + 2, :65],
                                    rhs,
                                    start=(t == 0),
                                    stop=(t == T - 2),
                                    perf_mode=DR,
                                )
                                if t == T - 2 and fin:
                                    drain(pt, qp)

                            av_jobs.append((g + 3, emit8))

                    emit_ready(g)
                    g += 1
            emit_ready(1 << 30)

    nc.compile()
    return nc


def _get_compiled():
    global _COMPILED
    if _COMPILED is None:
        _COMPILED = _build_program()
    return _COMPILED


def _mtiles(half):
    """Host-built mask tiles in PERM4 block order.

    mt4_8: Schraudolph multiplicative tile {M8, 0} for fp8 tail chunks.
    m01:   0/1 bf16 mask for slot 1 (applied to ACT real-exp output)."""
    ki = np.arange(KT_TILE)[:, None]
    qj = np.arange(QS)[None, :]
    d_a = (qj >= ki).astype(np.float32)
    d_b = (qj >= ki + 128).astype(np.float32)
    ones = np.ones((KT_TILE, QS), np.float32)
    zeros = np.zeros((KT_TILE, QS), np.float32)
    m4 = [ones, ones, d_a, d_b] if half == 0 else [d_a, d_b, zeros, zeros]

    def build(mscale):
        blocks = [None] * 4
        for r in range(4):
            blocks[PERM4[r]] = m4[r] * mscale
        return np.concatenate(blocks, axis=1).astype(BF16)

    return build(M8), build(1.0)


def make_in_maps(Q, K, V):
    """Pack full fp32 Q,K,V [B,S,D] into 8 per-core input dicts."""
    in_maps = []
    for c in range(NCORES):
        b = c % 4
        half = c // 4
        # KT packed: k-tile t -> partition half t%2, cols 128*(t//2)
        kt = np.ascontiguousarray(K[b].T)  # [64, 4096]
        ktp = np.empty((128, 16 * 128), np.float32)
        for t in range(NKT):
            h, u = t % 2, t // 2
            ktp[64 * h : 64 * h + 64, 128 * u : 128 * u + 128] = kt[
                :, 128 * t : 128 * t + 128
            ]
        # Q slots (duplicated into both partition halves)
        qrows = np.concatenate(
            [Q[b, 256 * (2 * m - 1 - half) : 256 * (2 * m - 1 - half) + 256] for m in range(1, 9)],
            axis=0,
        )  # [2048, 64]
        qt = np.ascontiguousarray(qrows.T)  # [64, 2048]
        qtd = np.concatenate([qt, qt], axis=0)  # [128, 2048]
        # V augmented with ones column, padded to VW-stride tiles
        va = np.zeros((128, NKT * VW), np.float32)
        for t in range(NKT):
            va[:, VW * t : VW * t + 64] = V[b, 128 * t : 128 * t + 128, :]
            va[:, VW * t + 64] = 1.0
        va16 = np.empty((128, 4 * 65), np.float32)
        for t in range(4):
            va16[:, 65 * t : 65 * t + 64] = V[b, 128 * t : 128 * t + 128, :]
            va16[:, 65 * t + 64] = 1.0
        mt4_8, m01 = _mtiles(half)
        in_maps.append(
            {
                "ktp": ktp.astype(BF16),
                "qtd": qtd.astype(BF16),
                "va8": va.astype(E4M3),
                "va16": va16.astype(BF16),
                "mt4_8": mt4_8,
                "m01": m01,
            }
        )
    return in_maps


def unpack_outputs(results):
    """Combine 8 per-core OT [65, 2048] fp32 into full output [B,S,D]."""
    out = np.empty((B, S, D), np.float32)
    for c in range(NCORES):
        b = c % 4
        half = c // 4
        otc = results[c]["ot"]  # [65, 2048]
        for m in range(1, 9):
            j = 2 * m - 1 - half
            sl = otc[:, 256 * (m - 1) : 256 * m]  # [65, 256]
            out[b, 256 * j : 256 * j + 256, :] = (sl[:64] / sl[64:65]).T
    return out


def run_on_hw(in_maps, trace=False, trace_cores=None):
    from concourse.bass_utils import run_bass_kernel_spmd

    nc = _get_compiled()
    return run_bass_kernel_spmd(
        nc, in_maps, core_ids=list(range(NCORES)), trace=trace, trace_cores=trace_cores
    )


def kernel(Q, K, V):
    Q = np.asarray(Q, np.float32)
    K = np.asarray(K, np.float32)
    V = np.asarray(V, np.float32)
    res = run_on_hw(make_in_maps(Q, K, V), trace=False)
    return unpack_outputs(res.results)


# revision 10
# speedup vs baseline: 1.0536x; 1.0536x over previous
"""Causal MHA (B=4, S=4096, D=64, scale=1/sqrt(S)) on 8 trn2 NeuronCores.

Final (v7): dual-engine exp + fp8 DoubleRow AV + depth-3 chunk pipeline.
Measured 46.1us (baseline 54.6us); rel err 2.2e-3 (gate 2e-2).

Iteration history: v2 6-tile chunks depth-2 = 56.6us (engines stalled on
the 2-buffer PSUM rotation, PE went HAM-cold mid-run).  v3 4-tile chunks
depth-3 + slot-1 real-exp fix = 49.1us.  v4 measured-cost rebalance +
paired drains = 49.6us.  v5 slot reorder [8..3,1,2] + interleave penalty
(WARM_MMS=6 experiment failed: HAM gate never tripped, 55.3us).  v6 =
v5 with WARM_MMS=10 = 48.1us.  v7 AV deferral 2 chunks = 46.1us
(deferral 3 regressed: longer exposed tail).

Fixed overheads (measured, not removable at kernel level): ~10.6us
framework epilogue (per-engine semaphore-reset streams + final barrier),
~3.4us HAM clock-gate warm-up window, ~2.5us first-input DMA latency.

Strategy (identical SPMD program on all 8 cores; per-core data differs):
  - Sharding: core c -> batch b=c%4, half=c//4; 8 query slots of 256 rows,
    slot m covers q-block j=2m-1-half, iterates T=4m k-tiles of 128 keys;
    scoresT layout [k, q]; ones-column denominator; host divides.
  - Chunks of FOUR k-tiles, PSUM scores [128,1024] fp32 = 2 banks, THREE
    buffers (6 banks) + 2 AV accumulator banks = 8.  Depth-3 keeps the QK
    refill off the exp engines' critical path (v2's depth-2 stalled both
    engines every other chunk, PE went HAM-cold 15.6-29us).
  - QK: bf16 paired matmuls (adjacent k-tiles in partition halves 0/1,
    concurrent in disjoint PE row groups; measured 120ns/pair warm),
    perm [0,2,1,3] puts each pair in different PSUM banks.
  - exp (elementwise wall, 36864 cols/lane) split across TWO engines:
      ACT: real exp (scale folded), direct fp8e4 output (bit-exact RNE,
           probe-verified), ~925ns/chunk.
      DVE: Schraudolph bit-trick i8 = RNE((s + B8)*M8): the int8 bits ARE
           e4m3 of exp(s/64) (~2.5%/elem sawtooth, cancels in the softmax
           ratio).  Masked tail chunks fold the causal mask in FREE via
           scalar_tensor_tensor (s + B8) * Mtile, Mtile in {M8, 0}.
           ~1192ns/chunk.  fp32->int8 convert is exact RNE (probe).
    Greedy balance by modeled ns; tails forced to DVE; drains balanced.
  - AV: fp8e4 DoubleRow, TWO k-tiles per matmul ([128,2,65] x [128,2,256],
    V tiles padded to stride 80 for the %16 pair-step ISA rule).  Measured
    109ns/pair with LDWEIGHTS fully hidden.  exp tiles are [128,2,2,256]
    so pair i is the clean slice [:, :, i, :].
  - Precision: rows q<512 (slot 1) have concentrated attention (softmax
    over <=2 tiles): sawtooth/fp8 errors don't cancel there (v2 measured
    4e-2 at row 1).  Slot 1 uses ACT real exp -> bf16, a DVE 0/1-mask
    multiply, and bf16 AV.  Everything else fp8 (worst other block 7e-3
    abs vs gate 7e-2).
  - ACT Exp table preloaded during input DMAs (1.3us off critical path).
  - PE warm-up burst (N=512 matmuls) trips the HAM clock gate to 2.4GHz
    while DMAs land; steady-state PE duty ~45% keeps it warm.
Modeled walls: ACT ~20.5us, DVE ~20.7us, PE ~16.5us busy.
"""

import sys

sys.path.insert(0, "/opt/trn_rl_repo")

import numpy as np
import ml_dtypes

B, S, D = 4, 4096, 64
NCORES = 8
NSLOTS = 8          # query slots per core, 256 queries each
QS = 256            # queries per slot
KT_TILE = 128       # keys per k-tile
NKT = S // KT_TILE  # 32 k-tiles per batch
VW = 80             # padded V-tile width (65 data cols; 80 for %16 pair step)
CS = 4              # k-tiles per chunk
BF16 = ml_dtypes.bfloat16
E4M3 = ml_dtypes.float8_e4m3

_COMPILED = None

WARM_MMS = 10       # PE warm-up matmuls (N=512) to trip the HAM clock gate
_CACHE_BUST = 9     # bump to force a NEFF recompile

# Schraudolph constants: exp(s/64) ~= bitcast(round((s + Bc)*Mc)).
# Mc is the bf16-representable slope; -0.3 centers the piecewise-linear
# error (fitted offline, numerics_sim.py; RNE convert probe-verified).
_LN2 = float(np.log(2.0))
M8 = float(BF16(8.0 / (64.0 * _LN2)))       # 0.180664
B8 = (56.0 - 0.3) / M8

PERM4 = [0, 2, 1, 3]   # tile r -> PSUM block; QK pairs hit different banks


def _act_ns(fd):
    # measured on HW at FD=1024 (v3 trace): 1110ns
    return (fd + 310.0) / 1.2


def _dve_ns(fd):
    # measured on HW at FD=1024 (v3 trace): 1224ns
    return (fd + 155.0) / 0.96


def _build_program():
    import concourse.bacc as bacc
    import concourse.tile as tile
    import concourse.mybir as mybir

    F32 = mybir.dt.float32
    MBF16 = mybir.dt.bfloat16
    I8 = mybir.dt.int8
    F8 = mybir.dt.float8e4
    EXPF = mybir.ActivationFunctionType.Exp
    ALU = mybir.AluOpType
    DR = mybir.MatmulPerfMode.DoubleRow

    nc = bacc.Bacc("TRN2", target_bir_lowering=False, debug=False, num_devices=NCORES)

    ktp = nc.dram_tensor("ktp", [128, 16 * 128], MBF16, kind="ExternalInput").ap()
    qtd = nc.dram_tensor("qtd", [128, NSLOTS * QS], MBF16, kind="ExternalInput").ap()
    va8 = nc.dram_tensor("va8", [128, NKT * VW], F8, kind="ExternalInput").ap()
    va16 = nc.dram_tensor("va16", [128, 4 * 65], MBF16, kind="ExternalInput").ap()
    mt4_8 = nc.dram_tensor("mt4_8", [128, 4 * QS], MBF16, kind="ExternalInput").ap()
    m01 = nc.dram_tensor("m01", [128, 4 * QS], MBF16, kind="ExternalInput").ap()
    ot = nc.dram_tensor("ot", [65, NSLOTS * QS], F32, kind="ExternalOutput").ap()

    with tile.TileContext(nc) as tc:
        with (
            tc.tile_pool(name="ins", bufs=1) as ins,
            tc.tile_pool(name="work", bufs=4) as work,
            tc.tile_pool(name="outs", bufs=2) as outs,
            tc.tile_pool(name="ps", bufs=1, space="PSUM") as ps,
            tc.tile_pool(name="pso", bufs=1, space="PSUM") as pso,
        ):
            ktp_sb = ins.tile([128, 16 * 128], MBF16)
            qtd_sb = ins.tile([128, NSLOTS * QS], MBF16)
            va8_sb = ins.tile([128, NKT, VW], F8)
            va16_sb = ins.tile([128, 4, 65], MBF16)
            mt4_8_sb = ins.tile([128, 4 * QS], MBF16)
            m01_sb = ins.tile([128, 4 * QS], MBF16)
            warm_sb = ins.tile([128, 512], MBF16)
            tbl_sb = ins.tile([128, 8], F8)

            # PE warm-up burst: trips the HAM clock gate to 2.4 GHz while
            # the input DMAs stream.  Also preload the ACT Exp spline table
            # (1.3us) so the first real exp doesn't pay it.
            nc.gpsimd.memset(warm_sb, 0.25)
            nc.scalar.activation(tbl_sb, warm_sb[:, :8], EXPF, scale=1.0 / 64.0)
            p_warm = ps.tile([128, CS * QS], F32, tag="scores", bufs=3)
            for _ in range(WARM_MMS):
                nc.tensor.matmul(
                    p_warm[:, :512],
                    warm_sb[:, :128],
                    warm_sb[:, :512],
                    start=True,
                    stop=True,
                )

            # Input DMAs: split + ordered by first consumption.
            nc.sync.dma_start(out=qtd_sb[:, 7 * QS :], in_=qtd[:, 7 * QS :])  # slot 8
            nc.sync.dma_start(out=ktp_sb[:, :256], in_=ktp[:, :256])          # k-tiles 0-3
            nc.sync.dma_start(out=ktp_sb[:, 256:512], in_=ktp[:, 256:512])    # k-tiles 4-7
            nc.sync.dma_start(out=va8_sb[:, :8, :], in_=va8[:, : 8 * VW])     # v-tiles 0-7
            nc.sync.dma_start(out=ktp_sb[:, 512:896], in_=ktp[:, 512:896])    # k-tiles 8-13
            nc.sync.dma_start(out=ktp_sb[:, 896:], in_=ktp[:, 896:])
            nc.sync.dma_start(out=va8_sb[:, 8:, :], in_=va8[:, 8 * VW :])
            nc.sync.dma_start(out=mt4_8_sb, in_=mt4_8)
            nc.sync.dma_start(out=qtd_sb[:, 4 * QS : 7 * QS], in_=qtd[:, 4 * QS : 7 * QS])
            nc.sync.dma_start(out=qtd_sb[:, : 4 * QS], in_=qtd[:, : 4 * QS])
            nc.sync.dma_start(out=va16_sb[:, :, :], in_=va16)
            nc.sync.dma_start(out=m01_sb, in_=m01)

            eng_ns = {"act": 0.0, "dve": 0.0}  # modeled busy time

            def drain(po_pair, qlo_pair):
                # one [65,512] copy + DMA per slot PAIR
                o_sb = outs.tile([65, 2 * QS], F32, tag="drain")
                if eng_ns["act"] + _act_ns(2 * QS) <= eng_ns["dve"] + _dve_ns(2 * QS):
                    nc.scalar.copy(o_sb, po_pair)
                    eng_ns["act"] += _act_ns(2 * QS)
                else:
                    nc.vector.tensor_copy(o_sb, po_pair)
                    eng_ns["dve"] += _dve_ns(2 * QS)
                nc.sync.dma_start(out=ot[:, qlo_pair : qlo_pair + 2 * QS], in_=o_sb)

            # AV jobs deferred by one chunk so exp->AV is never on the
            # ACT/DVE critical path.  job: (ready_g, emit_fn)
            av_jobs = []

            def emit_ready(now):
                rest = []
                for ready, fn in av_jobs:
                    if ready <= now:
                        fn()
                    else:
                        rest.append((ready, fn))
                av_jobs[:] = rest

            FD = CS * QS
            g = 0  # global chunk counter
            po_pair = None
            prev_eng = [None]  # engine of the previous chunk, for interleaving
            SLOT_ORDER = [8, 7, 6, 5, 4, 3, 1, 2]
            for oi, m in enumerate(SLOT_ORDER):
                qlo = (m - 1) * QS
                T = 4 * m
                slot1 = m == 1
                if oi % 2 == 0:  # first slot of a pair
                    po_pair = pso.tile([65, 2 * QS], F32, tag="avout", bufs=2)
                    pair_lo = min(m, SLOT_ORDER[oi + 1])
                # within the pair bank: smaller-m slot -> cols 0:256 (matches
                # the contiguous ot range of the pair)
                p_out = po_pair[:, :QS] if m == pair_lo else po_pair[:, QS : 2 * QS]
                last_of_pair = oi % 2 == 1
                pair_tile = po_pair
                qlo_pair = (pair_lo - 1) * QS if last_of_pair else None
                nchunks = T // CS
                for ci in range(nchunks):
                    base = ci * CS
                    tail = ci == nchunks - 1
                    p_sc = ps.tile([128, CS * QS], F32, tag="scores", bufs=3)
                    for j in range(CS):
                        t = base + j
                        h = t % 2
                        u = t // 2
                        nc.tensor.matmul(
                            p_sc[:, PERM4[j] * QS : PERM4[j] * QS + QS],
                            ktp_sb[64 * h : 64 * h + 64, 128 * u : 128 * u + 128],
                            qtd_sb[64 * h : 64 * h + 64, qlo : qlo + QS],
                            start=True,
                            stop=True,
                        )

                    # ---- exp + AV jobs ----
                    if slot1:
                        # concentrated-attention rows: real exp, bf16 AV.
                        e16 = work.tile([128, 2, 2, QS], MBF16, tag="e16")
                        nc.scalar.activation(
                            e16[:, :, :, :], p_sc[:, :FD], EXPF, scale=1.0 / 64.0
                        )
                        eng_ns["act"] += _act_ns(FD)
                        prev_eng[0] = "act"
                        e16m = work.tile([128, 2, 2, QS], MBF16, tag="e16m")
                        nc.gpsimd.tensor_mul(
                            e16m[:, :, :, :], e16[:, :, :, :], m01_sb
                        )
                        for j in range(CS):
                            t = base + j
                            r, i = j % 2, j // 2

                            def emit16(t=t, r=r, i=i, e=e16m, po=p_out, T=T,
                                       fin=last_of_pair, pt=pair_tile, qp=qlo_pair):
                                nc.tensor.matmul(
                                    po,
                                    va16_sb[:, t, :],
                                    e[:, r, i, :],
                                    start=(t == 0),
                                    stop=(t == T - 1),
                                )
                                if t == T - 1 and fin:
                                    drain(pt, qp)

                            av_jobs.append((g + 2, emit16))
                    else:
                        if tail:
                            # masked fp8 Schraudolph on DVE, mask fused
                            e8 = work.tile([128, 2, 2, QS], I8, tag="e8i")
                            nc.vector.scalar_tensor_tensor(
                                e8[:, :, :, :], p_sc[:, :FD], float(B8),
                                mt4_8_sb, ALU.add, ALU.mult,
                            )
                            eng_ns["dve"] += _dve_ns(FD)
                            prev_eng[0] = "dve"
                            nat = False
                        elif m == 2 or (eng_ns["act"] + _act_ns(FD)
                              + (600.0 if prev_eng[0] == "act" else 0.0)) <= (
                              eng_ns["dve"] + _dve_ns(FD)
                              + (600.0 if prev_eng[0] == "dve" else 0.0)):
                            e8 = work.tile([128, 2, 2, QS], F8, tag="e8a")
                            nc.scalar.activation(
                                e8[:, :, :, :], p_sc[:, :FD], EXPF, scale=1.0 / 64.0
                            )
                            eng_ns["act"] += _act_ns(FD)
                            prev_eng[0] = "act"
                            nat = True
                        else:
                            e8 = work.tile([128, 2, 2, QS], I8, tag="e8i")
                            nc.vector.tensor_scalar(
                                e8[:, :, :, :], p_sc[:, :FD], float(B8), float(M8),
                                ALU.add, ALU.mult,
                            )
                            eng_ns["dve"] += _dve_ns(FD)
                            prev_eng[0] = "dve"
                            nat = False

                        for i in range(2):
                            t = base + 2 * i

                            def emit8(t=t, i=i, e=e8, nat=nat, po=p_out, T=T,
                                      fin=last_of_pair, pt=pair_tile, qp=qlo_pair):
                                rhs = e[:, :, i, :]
                                if not nat:
                                    rhs = rhs.bitcast(F8)
                                nc.tensor.matmul(
                                    po,
                                    va8_sb[:, t : t + 2, :65],
                                    rhs,
                                    start=(t == 0),
                                    stop=(t == T - 2),
                                    perf_mode=DR,
                                )
                                if t == T - 2 and fin:
                                    drain(pt, qp)

                            av_jobs.append((g + 2, emit8))

                    emit_ready(g)
                    g += 1
            emit_ready(1 << 30)

    nc.compile()
    return nc


def _get_compiled():
    global _COMPILED
    if _COMPILED is None:
        _COMPILED = _build_program()
    return _COMPILED


def _mtiles(half):
    """Host-built mask tiles in PERM4 block order.

    mt4_8: Schraudolph multiplicative tile {M8, 0} for fp8 tail chunks.
    m01:   0/1 bf16 mask for slot 1 (applied to ACT real-exp output)."""
    ki = np.arange(KT_TILE)[:, None]
    qj = np.arange(QS)[None, :]
    d_a = (qj >= ki).astype(np.float32)
    d_b = (qj >= ki + 128).astype(np.float32)
    ones = np.ones((KT_TILE, QS), np.float32)
    zeros = np.zeros((KT_TILE, QS), np.float32)
    m4 = [ones, ones, d_a, d_b] if half == 0 else [d_a, d_b, zeros, zeros]

    def build(mscale):
        blocks = [None] * 4
        for r in range(4):
            blocks[PERM4[r]] = m4[r] * mscale
        return np.concatenate(blocks, axis=1).astype(BF16)

    return build(M8), build(1.0)


def make_in_maps(Q, K, V):
    """Pack full fp32 Q,K,V [B,S,D] into 8 per-core input dicts."""
    in_maps = []
    for c in range(NCORES):
        b = c % 4
        half = c // 4
        # KT packed: k-tile t -> partition half t%2, cols 128*(t//2)
        kt = np.ascontiguousarray(K[b].T)  # [64, 4096]
        ktp = np.empty((128, 16 * 128), np.float32)
        for t in range(NKT):
            h, u = t % 2, t // 2
            ktp[64 * h : 64 * h + 64, 128 * u : 128 * u + 128] = kt[
                :, 128 * t : 128 * t + 128
            ]
        # Q slots (duplicated into both partition halves)
        qrows = np.concatenate(
            [Q[b, 256 * (2 * m - 1 - half) : 256 * (2 * m - 1 - half) + 256] for m in range(1, 9)],
            axis=0,
        )  # [2048, 64]
        qt = np.ascontiguousarray(qrows.T)  # [64, 2048]
        qtd = np.concatenate([qt, qt], axis=0)  # [128, 2048]
        # V augmented with ones column, padded to VW-stride tiles
        va = np.zeros((128, NKT * VW), np.float32)
        for t in range(NKT):
            va[:, VW * t : VW * t + 64] = V[b, 128 * t : 128 * t + 128, :]
            va[:, VW * t + 64] = 1.0
        va16 = np.empty((128, 4 * 65), np.float32)
        for t in range(4):
            va16[:, 65 * t : 65 * t + 64] = V[b, 128 * t : 128 * t + 128, :]
            va16[:, 65 * t + 64] = 1.0
        mt4_8, m01 = _mtiles(half)
        in_maps.append(
            {
                "ktp": ktp.astype(BF16),
                "qtd": qtd.astype(BF16),
                "va8": va.astype(E4M3),
                "va16": va16.astype(BF16),
                "mt4_8": mt4_8,
                "m01": m01,
            }
        )
    return in_maps


def unpack_outputs(results):
    """Combine 8 per-core OT [65, 2048] fp32 into full output [B,S,D]."""
    out = np.empty((B, S, D), np.float32)
    for c in range(NCORES):
        b = c % 4
        half = c // 4
        otc = results[c]["ot"]  # [65, 2048]
        for m in range(1, 9):
            j = 2 * m - 1 - half
            sl = otc[:, 256 * (m - 1) : 256 * m]  # [65, 256]
            out[b, 256 * j : 256 * j + 256, :] = (sl[:64] / sl[64:65]).T
    return out


def run_on_hw(in_maps, trace=False, trace_cores=None):
    from concourse.bass_utils import run_bass_kernel_spmd

    nc = _get_compiled()
    return run_bass_kernel_spmd(
        nc, in_maps, core_ids=list(range(NCORES)), trace=trace, trace_cores=trace_cores
    )


def kernel(Q, K, V):
    Q = np.asarray(Q, np.float32)
    K = np.asarray(K, np.float32)
    V = np.asarray(V, np.float32)
    res = run_on_hw(make_in_maps(Q, K, V), trace=False)
    return unpack_outputs(res.results)
